# revision 4
# baseline (speedup 1.0000x reference)
"""Cross-attention (B=4, T=S=1024, C=1024, H=16, D=64) on 8 trn2 NeuronCores.

Sharding: core c handles batch b=c//2, sequence half hf=c%2 (512 q-rows).
k/v are computed for the core's own 512 encoder rows and exchanged within
the (2b, 2b+1) pair via AllGather. All activations are kept channel-major
("transposed", [C, T]-style) on chip so no transposes are ever needed; the
host transposes the per-core inputs/outputs (cheap numpy .T copies).

Per-core pipeline (everything fp32r on the PE, ~1e-4 matmul error):
  1. qT = (x Wq + bq)^T, kT likewise, v = enc Wv + bv (natural [s, c] layout,
     stored per-head padded [s, 16, 65] with a ones column at slot 64).
     RMSNorm+residual on q/k: per-token rsqrt(mean(q^2)) via ones-matmul
     column reduction + K=1 outer-product broadcast of (1 + scale*rr).
  2. Pairwise AllGather of kT [1024,512] and v_aug [512,1040].
  3. Per head h: scoresT[s-tile, t] = kh^T.T @ qh^T (K=64; head pairs run
     concurrently on PE row groups), exp on ACT (scale=1/8), then
     y_aug[65,512] = sum_s [v_h | 1].T @ exp  -- row 64 is the softmax
     denominator Z. rb = outer(1/16, 1/Z) via K=1 matmul; attn/16 = exp*rb
     accumulates into attn_mean; yT = 16 * y_aug[0:64] * rb.
  4. youtT = (yT Wp + bp)^T, DMA out; host transposes + reassembles.
"""

import numpy as np

import concourse.bacc as bacc
import concourse.mybir as mybir
import concourse.tile as tile
from concourse.bass_utils import run_bass_kernel_spmd

F32 = mybir.dt.float32
F32R = mybir.dt.float32r
AF = mybir.ActivationFunctionType
ALU = mybir.AluOpType

B, T, S, C, H = 4, 1024, 1024, 1024, 16
D = C // H            # 64
TN = 512              # per-core q rows / kv rows
KT = 8                # contraction tiles (C/128)
MT = 8                # output-channel tiles
ST = 8                # global s tiles (S/128)
DA = D + 1            # augmented head width (ones column at 64)
GROUPS = [[0, 1], [2, 3], [4, 5], [6, 7]]


def build():
    nc = bacc.Bacc("TRN2", target_bir_lowering=False, debug=False, num_devices=8)

    xT_d = nc.dram_tensor("xT", [C, TN], F32R, kind="ExternalInput")
    encT_d = nc.dram_tensor("encT", [C, TN], F32R, kind="ExternalInput")
    wq_d = nc.dram_tensor("wq", [C, C], F32R, kind="ExternalInput")
    wk_d = nc.dram_tensor("wk", [C, C], F32R, kind="ExternalInput")
    wv_d = nc.dram_tensor("wv", [C, C], F32R, kind="ExternalInput")
    wp_d = nc.dram_tensor("wp", [C, C], F32R, kind="ExternalInput")
    bq_d = nc.dram_tensor("bq", [128, MT], F32, kind="ExternalInput")
    bk_d = nc.dram_tensor("bk", [128, MT], F32, kind="ExternalInput")
    bp_d = nc.dram_tensor("bp", [128, MT], F32, kind="ExternalInput")
    bv_d = nc.dram_tensor("bv", [C], F32, kind="ExternalInput")
    qs_d = nc.dram_tensor("qs", [C], F32R, kind="ExternalInput")
    ks_d = nc.dram_tensor("ks", [C], F32R, kind="ExternalInput")

    yT_o = nc.dram_tensor("youtT", [C, TN], F32, kind="ExternalOutput")
    am_o = nc.dram_tensor("ameanT", [S, TN], F32, kind="ExternalOutput")

    k_bounce = nc.dram_tensor("k_bounce", [C, TN], F32R)
    kg = nc.dram_tensor("kg", [2, C, TN], F32R)
    v_bounce = nc.dram_tensor("v_bounce", [TN, H * DA], F32R)
    vg = nc.dram_tensor("vg", [2, TN, H * DA], F32R)

    with tile.TileContext(nc) as tc:
        with (
            tc.tile_pool(name="const", bufs=1) as cst,
            tc.tile_pool(name="qt", bufs=1) as qt_pool,
            tc.tile_pool(name="acc", bufs=1) as acc_pool,
            tc.tile_pool(name="yt", bufs=1) as yt_pool,
        ):
            # ---- constants ----
            ones_col_f = cst.tile([128, 1], F32)
            nc.vector.memset(ones_col_f[:], 1.0)
            ones_col = cst.tile([128, 1], F32R)
            nc.vector.tensor_copy(ones_col[:], ones_col_f[:])
            inv16_row_f = cst.tile([1, 128], F32)
            nc.vector.memset(inv16_row_f[:], 1.0 / 16.0)
            inv16_row = cst.tile([1, 128], F32R)
            nc.vector.tensor_copy(inv16_row[:], inv16_row_f[:])
            ones_hf = cst.tile([128, H], F32)
            nc.vector.memset(ones_hf[:], 1.0)
            eps_t = cst.tile([1, 1], F32)
            nc.vector.memset(eps_t[:], 1e-6)

            bq_sb = cst.tile([128, MT], F32)
            bk_sb = cst.tile([128, MT], F32)
            bp_sb = cst.tile([128, MT], F32)
            nc.sync.dma_start(bq_sb[:], bq_d.ap())
            nc.sync.dma_start(bk_sb[:], bk_d.ap())
            nc.sync.dma_start(bp_sb[:], bp_d.ap())
            bv_bc = cst.tile([128, C], F32)
            nc.sync.dma_start(bv_bc[:], bv_d.ap().partition_broadcast(128))
            qs_sb = cst.tile([1, C], F32R)
            ks_sb = cst.tile([1, C], F32R)
            nc.sync.dma_start(qs_sb[:], qs_d.ap().unsqueeze(0))
            nc.sync.dma_start(ks_sb[:], ks_d.ap().unsqueeze(0))

            qT = [qt_pool.tile([128, TN], F32R, tag=f"qT{m}", name=f"qT{m}") for m in range(MT)]
            acc = [acc_pool.tile([128, TN], F32, tag=f"acc{j}", name=f"acc{j}") for j in range(ST)]
            yT = [yt_pool.tile([128, TN], F32R, tag=f"yT{k}", name=f"yT{k}") for k in range(KT)]

            # ================= phase 1: projections =================
            with (
                tc.tile_pool(name="p1_in", bufs=1) as p1_in,
                tc.tile_pool(name="p1_w", bufs=1) as p1_w,
                tc.tile_pool(name="p1_kv", bufs=1) as p1_kv,
                tc.tile_pool(name="p1_sq", bufs=2) as p1_sq,
                tc.tile_pool(name="p1_ps", bufs=2, space="PSUM") as p1_ps,
                tc.tile_pool(name="p1_ss", bufs=1, space="PSUM") as p1_ss,
                tc.tile_pool(name="p1_f", bufs=2, space="PSUM") as p1_f,
            ):
                xT = [p1_in.tile([128, TN], F32R, tag=f"xT{k}", name=f"xTs{k}") for k in range(KT)]
                eT = [p1_in.tile([128, TN], F32R, tag=f"eT{k}", name=f"eTs{k}") for k in range(KT)]
                for k in range(KT):
                    nc.sync.dma_start(xT[k][:], xT_d.ap()[k * 128 : (k + 1) * 128, :])
                    nc.sync.dma_start(eT[k][:], encT_d.ap()[k * 128 : (k + 1) * 128, :])

                def qk_projection(w_d, in_tiles, bias_sb, scale_sb, out_tiles):
                    w_sb = p1_w.tile([128, KT, C], F32R, tag="W")
                    for k in range(KT):
                        nc.sync.dma_start(
                            w_sb[:, k, :], w_d.ap()[k * 128 : (k + 1) * 128, :]
                        )
                    ssum = p1_ss.tile([1, TN], F32, tag="ssum")
                    for m in range(MT):
                        ps = p1_ps.tile([128, TN], F32, tag="proj")
                        for k in range(KT):
                            nc.tensor.matmul(
                                ps[:],
                                w_sb[:, k, m * 128 : (m + 1) * 128],
                                in_tiles[k][:],
                                start=(k == 0),
                                stop=(k == KT - 1),
                            )
                        # bias add (psum f32 -> sbuf f32r)
                        nc.vector.tensor_scalar_add(
                            out_tiles[m][:], ps[:], bias_sb[:, m : m + 1]
                        )
                        sq = p1_sq.tile([128, TN], F32R, tag="sq")
                        nc.scalar.activation(sq[:], out_tiles[m][:], AF.Square)
                        nc.tensor.matmul(
                            ssum[:],
                            ones_col[:],
                            sq[:],
                            start=(m == 0),
                            stop=(m == MT - 1),
                        )
                    # rr = 1/sqrt(ssum/C + eps)
                    rms = p1_sq.tile([1, TN], F32, tag="rms")
                    nc.scalar.activation(
                        rms[:], ssum[:], AF.Sqrt, scale=1.0 / C, bias=eps_t[:]
                    )
                    rr = p1_sq.tile([1, TN], F32R, tag="rr")
                    with nc.allow_low_precision(reason="rms rsqrt broadcast"):
                        nc.vector.reciprocal(rr[:], rms[:])
                    for m in range(MT):
                        fps = p1_f.tile([128, TN], F32, tag="fps")
                        nc.tensor.matmul(
                            fps[:],
                            scale_sb[:, m * 128 : (m + 1) * 128],
                            rr[:],
                            start=True,
                            stop=True,
                        )
                        f1 = p1_sq.tile([128, TN], F32R, tag="f1")
                        nc.scalar.activation(f1[:], fps[:], AF.Copy, bias=1.0)
                        nc.vector.tensor_mul(out_tiles[m][:], out_tiles[m][:], f1[:])

                # q projection
                qk_projection(wq_d, xT, bq_sb, qs_sb, qT)

                # k projection -> kT tiles then bounce out
                kT = [p1_kv.tile([128, TN], F32R, tag=f"kT{m}", name=f"kTs{m}") for m in range(MT)]
                qk_projection(wk_d, eT, bk_sb, ks_sb, kT)
                for m in range(MT):
                    nc.sync.dma_start(
                        k_bounce.ap()[m * 128 : (m + 1) * 128, :], kT[m][:]
                    )

                # v projection: natural [s, c] layout, per-head padded + ones col
                v_loc = [
                    p1_kv.tile([128, H, DA], F32R, tag=f"vl{st}", name=f"vl{st}") for st in range(4)
                ]
                wv_sb = p1_w.tile([128, KT, C], F32R, tag="W")
                for k in range(KT):
                    nc.sync.dma_start(
                        wv_sb[:, k, :], wv_d.ap()[k * 128 : (k + 1) * 128, :]
                    )
                for st in range(4):
                    for jh in range(2):
                        ps = p1_ps.tile([128, TN], F32, tag="proj")
                        for k in range(KT):
                            nc.tensor.matmul(
                                ps[:],
                                eT[k][:, st * 128 : (st + 1) * 128],
                                wv_sb[:, k, jh * 512 : (jh + 1) * 512],
                                start=(k == 0),
                                stop=(k == KT - 1),
                            )
                        nc.vector.tensor_add(
                            v_loc[st][:, jh * 8 : (jh + 1) * 8, 0:D],
                            ps[:].rearrange("p (h d) -> p h d", h=8),
                            bv_bc[:, jh * 512 : (jh + 1) * 512].rearrange(
                                "p (h d) -> p h d", h=8
                            ),
                        )
                    nc.vector.tensor_copy(v_loc[st][:, :, D], ones_hf[:])
                    nc.sync.dma_start(
                        v_bounce.ap()[st * 128 : (st + 1) * 128, :],
                        v_loc[st][:].rearrange("p h d -> p (h d)"),
                    )

            # ================= phase 2: pairwise allgather =================
            nc.gpsimd.collective_compute(
                "AllGather",
                ALU.bypass,
                replica_groups=GROUPS,
                ins=[k_bounce.ap()],
                outs=[kg.ap()],
            )
            nc.gpsimd.collective_compute(
                "AllGather",
                ALU.bypass,
                replica_groups=GROUPS,
                ins=[v_bounce.ap()],
                outs=[vg.ap()],
            )

            # ================= phase 3: attention =================
            with (
                tc.tile_pool(name="p3_k", bufs=1) as p3_k,
                tc.tile_pool(name="p3_v", bufs=1) as p3_v,
                tc.tile_pool(name="p3_e", bufs=2) as p3_e,
                tc.tile_pool(name="p3_rb", bufs=2) as p3_rb,
                tc.tile_pool(name="p3_sc", bufs=3, space="PSUM") as p3_sc,
                tc.tile_pool(name="p3_y", bufs=2, space="PSUM") as p3_y,
                tc.tile_pool(name="p3_rp", bufs=2, space="PSUM") as p3_rp,
            ):
                kTf = [p3_k.tile([128, 2, TN], F32R, tag=f"kTf{m}", name=f"kTf{m}") for m in range(MT)]
                for m in range(MT):
                    nc.sync.dma_start(
                        kTf[m][:],
                        kg.ap()[:, m * 128 : (m + 1) * 128, :].transpose([1, 0, 2]),
                    )
                vf = [p3_v.tile([128, H, DA], F32R, tag=f"vf{j}", name=f"vf{j}") for j in range(ST)]
                for j in range(ST):
                    nc.sync.dma_start(
                        vf[j][:],
                        vg.ap()[j // 4, (j % 4) * 128 : (j % 4 + 1) * 128, :].rearrange(
                            "p (h d) -> p h d", h=H
                        ),
                    )

                for h in range(H):
                    mt, base = h // 2, (h % 2) * 64
                    qh = qT[mt][base : base + 64, :]
                    exps = []
                    for j in range(ST):
                        sc = p3_sc.tile([128, TN], F32, tag="sc")
                        nc.tensor.matmul(
                            sc[:],
                            kTf[mt][base : base + 64, j // 4,
                                    (j % 4) * 128 : (j % 4 + 1) * 128],
                            qh,
                            start=True,
                            stop=True,
                        )
                        ex = p3_e.tile([128, TN], F32R, tag=f"exp{j}")
                        nc.scalar.activation(ex[:], sc[:], AF.Exp, scale=float(D) ** -0.5)
                        exps.append(ex)
                    y_ps = p3_y.tile([DA, TN], F32, tag="y")
                    for j in range(ST):
                        nc.tensor.matmul(
                            y_ps[:],
                            vf[j][:, h, :],
                            exps[j][:],
                            start=(j == 0),
                            stop=(j == ST - 1),
                        )
                    recip = p3_rb.tile([1, TN], F32R, tag="recip")
                    with nc.allow_low_precision(reason="softmax 1/Z broadcast"):
                        nc.vector.reciprocal(recip[:], y_ps[64:65, :])
                    rb_ps = p3_rp.tile([128, TN], F32, tag="rb")
                    nc.tensor.matmul(
                        rb_ps[:], inv16_row[:], recip[:], start=True, stop=True
                    )
                    # yT[c-tile mt, rows base:base+64] = 16 * y_aug * rb
                    rb_sb = p3_rb.tile([128, TN], F32, tag="rbsb")
                    nc.scalar.activation(rb_sb[:], rb_ps[:], AF.Copy)
                    nc.vector.scalar_tensor_tensor(
                        yT[mt][base : base + 64, :],
                        y_ps[0:64, :],
                        16.0,
                        rb_sb[0:64, :],
                        ALU.mult,
                        ALU.mult,
                    )
                    # attn_mean accumulation: acc[j] += exp[j] * rb  (rb = 1/(16 Z))
                    for j in range(ST):
                        if h == 0:
                            nc.vector.tensor_mul(acc[j][:], exps[j][:], rb_ps[:])
                        else:
                            nc.vector.tensor_mul(exps[j][:], exps[j][:], rb_ps[:])
                            nc.vector.tensor_add(
                                acc[j][:], acc[j][:], exps[j][:].bitcast(F32)
                            )

            for j in range(ST):
                nc.sync.dma_start(am_o.ap()[j * 128 : (j + 1) * 128, :], acc[j][:])

            # ================= phase 4: output projection =================
            with (
                tc.tile_pool(name="p4_w", bufs=1) as p4_w,
                tc.tile_pool(name="p4_o", bufs=2) as p4_o,
                tc.tile_pool(name="p4_ps", bufs=2, space="PSUM") as p4_ps,
            ):
                wp_sb = p4_w.tile([128, KT, C], F32R, tag="Wp")
                for k in range(KT):
                    nc.sync.dma_start(
                        wp_sb[:, k, :], wp_d.ap()[k * 128 : (k + 1) * 128, :]
                    )
                for m in range(MT):
                    ps = p4_ps.tile([128, TN], F32, tag="yo")
                    for k in range(KT):
                        nc.tensor.matmul(
                            ps[:],
                            wp_sb[:, k, m * 128 : (m + 1) * 128],
                            yT[k][:],
                            start=(k == 0),
                            stop=(k == KT - 1),
                        )
                    yo = p4_o.tile([128, TN], F32, tag="yo_sb")
                    nc.vector.tensor_scalar_add(yo[:], ps[:], bp_sb[:, m : m + 1])
                    nc.sync.dma_start(yT_o.ap()[m * 128 : (m + 1) * 128, :], yo[:])

    nc.compile()
    return nc


_NC_CACHE = None


def _get_nc():
    global _NC_CACHE
    if _NC_CACHE is None:
        _NC_CACHE = build()
    return _NC_CACHE


def kernel(x, encoder_output, Wq, bq, Wk, bk, Wv, bv, q_scale, k_scale, Wp, bp,
           _trace=False):
    x = np.asarray(x, np.float32)
    enc = np.asarray(encoder_output, np.float32)
    Wq = np.ascontiguousarray(np.asarray(Wq, np.float32))
    Wk = np.ascontiguousarray(np.asarray(Wk, np.float32))
    Wv = np.ascontiguousarray(np.asarray(Wv, np.float32))
    Wp = np.ascontiguousarray(np.asarray(Wp, np.float32))
    bq_t = np.ascontiguousarray(np.asarray(bq, np.float32).reshape(MT, 128).T)
    bk_t = np.ascontiguousarray(np.asarray(bk, np.float32).reshape(MT, 128).T)
    bp_t = np.ascontiguousarray(np.asarray(bp, np.float32).reshape(MT, 128).T)
    bv = np.ascontiguousarray(np.asarray(bv, np.float32))
    qs = np.ascontiguousarray(np.asarray(q_scale, np.float32))
    ks = np.ascontiguousarray(np.asarray(k_scale, np.float32))

    in_maps = []
    for c in range(8):
        b, hf = c // 2, c % 2
        xT = np.ascontiguousarray(x[b, hf * TN : (hf + 1) * TN, :].T)
        encT = np.ascontiguousarray(enc[b, hf * TN : (hf + 1) * TN, :].T)
        in_maps.append(
            dict(xT=xT, encT=encT, wq=Wq, wk=Wk, wv=Wv, wp=Wp,
                 bq=bq_t, bk=bk_t, bp=bp_t, bv=bv, qs=qs, ks=ks)
        )

    nc = _get_nc()
    res = run_bass_kernel_spmd(nc, in_maps, core_ids=list(range(8)), trace=_trace)

    y = np.empty((B, T, C), np.float32)
    amean = np.empty((B, T, S), np.float32)
    for c in range(8):
        b, hf = c // 2, c % 2
        r = res.results[c]
        y[b, hf * TN : (hf + 1) * TN, :] = r["youtT"].T
        amean[b, hf * TN : (hf + 1) * TN, :] = r["ameanT"].T
    if _trace:
        kernel.last_exec_time_ns = res.exec_time_ns
        kernel.last_results = res
    return y, amean


# revision 5
# speedup vs baseline: 7065.2140x; 7065.2140x over previous
"""Cross-attention (B=4, T=S=1024, C=1024, H=16, D=64) on 8 trn2 NeuronCores.

Sharding: core c handles batch b=c//2, sequence half hf=c%2 (512 q-rows).
k/v are computed for the core's own 512 encoder rows and exchanged within
the (2b, 2b+1) pair via AllGather. All activations are kept channel-major
("transposed", [C, T]-style) on chip so no transposes are ever needed; the
host transposes the per-core inputs/outputs (cheap numpy .T copies).

Per-core pipeline (everything fp32r on the PE, ~1e-4 matmul error):
  1. qT = (x Wq + bq)^T, kT likewise, v = enc Wv + bv (natural [s, c] layout,
     stored per-head padded [s, 16, 65] with a ones column at slot 64).
     RMSNorm+residual on q/k: per-token rsqrt(mean(q^2)) via ones-matmul
     column reduction + K=1 outer-product broadcast of (1 + scale*rr).
  2. Pairwise AllGather of kT [1024,512] and v_aug [512,1040].
  3. Per head h: scoresT[s-tile, t] = kh^T.T @ qh^T (K=64; head pairs run
     concurrently on PE row groups), exp on ACT (scale=1/8), then
     y_aug[65,512] = sum_s [v_h | 1].T @ exp  -- row 64 is the softmax
     denominator Z. rb = outer(1/16, 1/Z) via K=1 matmul; attn/16 = exp*rb
     accumulates into attn_mean; yT = 16 * y_aug[0:64] * rb.
  4. youtT = (yT Wp + bp)^T, DMA out; host transposes + reassembles.
"""

import numpy as np

import concourse.bacc as bacc
import concourse.mybir as mybir
import concourse.tile as tile
from concourse.bass_utils import run_bass_kernel_spmd

F32 = mybir.dt.float32
F32R = mybir.dt.float32r
AF = mybir.ActivationFunctionType
ALU = mybir.AluOpType

B, T, S, C, H = 4, 1024, 1024, 1024, 16
D = C // H            # 64
TN = 512              # per-core q rows / kv rows
KT = 8                # contraction tiles (C/128)
MT = 8                # output-channel tiles
ST = 8                # global s tiles (S/128)
DA = D + 1            # augmented head width (ones column at 64)
GROUPS = [[0, 1], [2, 3], [4, 5], [6, 7]]


def build():
    nc = bacc.Bacc("TRN2", target_bir_lowering=False, debug=False, num_devices=8)

    xT_d = nc.dram_tensor("xT", [C, TN], F32R, kind="ExternalInput")
    encT_d = nc.dram_tensor("encT", [C, TN], F32R, kind="ExternalInput")
    wq_d = nc.dram_tensor("wq", [C, C], F32R, kind="ExternalInput")
    wk_d = nc.dram_tensor("wk", [C, C], F32R, kind="ExternalInput")
    wv_d = nc.dram_tensor("wv", [C, C], F32R, kind="ExternalInput")
    wp_d = nc.dram_tensor("wp", [C, C], F32R, kind="ExternalInput")
    bq_d = nc.dram_tensor("bq", [128, MT], F32, kind="ExternalInput")
    bk_d = nc.dram_tensor("bk", [128, MT], F32, kind="ExternalInput")
    bp_d = nc.dram_tensor("bp", [128, MT], F32, kind="ExternalInput")
    bv_d = nc.dram_tensor("bv", [C], F32, kind="ExternalInput")
    qs_d = nc.dram_tensor("qs", [C], F32R, kind="ExternalInput")
    ks_d = nc.dram_tensor("ks", [C], F32R, kind="ExternalInput")

    yT_o = nc.dram_tensor("youtT", [C, TN], F32, kind="ExternalOutput")
    am_o = nc.dram_tensor("ameanT", [S, TN], F32, kind="ExternalOutput")

    k_bounce = nc.dram_tensor("k_bounce", [C, TN], F32R)
    kg = nc.dram_tensor("kg", [2, C, TN], F32R)
    v_bounce = nc.dram_tensor("v_bounce", [TN, H * DA], F32R)
    vg = nc.dram_tensor("vg", [2, TN, H * DA], F32R)

    with tile.TileContext(nc) as tc:
        with (
            tc.tile_pool(name="const", bufs=1) as cst,
            tc.tile_pool(name="qt", bufs=1) as qt_pool,
            tc.tile_pool(name="acc", bufs=1) as acc_pool,
            tc.tile_pool(name="yt", bufs=1) as yt_pool,
        ):
            # ---- constants ----
            ones_col_f = cst.tile([128, 1], F32)
            nc.vector.memset(ones_col_f[:], 1.0)
            ones_col = cst.tile([128, 1], F32R)
            nc.vector.tensor_copy(ones_col[:], ones_col_f[:])
            inv16_row_f = cst.tile([1, 128], F32)
            nc.vector.memset(inv16_row_f[:], 1.0 / 16.0)
            inv16_row = cst.tile([1, 128], F32R)
            nc.vector.tensor_copy(inv16_row[:], inv16_row_f[:])
            ones_hf = cst.tile([128, H], F32)
            nc.vector.memset(ones_hf[:], 1.0)
            eps_t = cst.tile([1, 1], F32)
            nc.vector.memset(eps_t[:], 1e-6)

            bq_sb = cst.tile([128, MT], F32)
            bk_sb = cst.tile([128, MT], F32)
            bp_sb = cst.tile([128, MT], F32)
            nc.sync.dma_start(bq_sb[:], bq_d.ap())
            nc.sync.dma_start(bk_sb[:], bk_d.ap())
            nc.sync.dma_start(bp_sb[:], bp_d.ap())
            bv_bc = cst.tile([128, C], F32)
            nc.sync.dma_start(bv_bc[:], bv_d.ap().partition_broadcast(128))
            qs_sb = cst.tile([1, C], F32R)
            ks_sb = cst.tile([1, C], F32R)
            nc.sync.dma_start(qs_sb[:], qs_d.ap().unsqueeze(0))
            nc.sync.dma_start(ks_sb[:], ks_d.ap().unsqueeze(0))

            qT = [qt_pool.tile([128, TN], F32R, tag=f"qT{m}", name=f"qT{m}") for m in range(MT)]
            acc = [acc_pool.tile([128, TN], F32, tag=f"acc{j}", name=f"acc{j}") for j in range(ST)]
            yT = [yt_pool.tile([128, TN], F32R, tag=f"yT{k}", name=f"yT{k}") for k in range(KT)]

            # ================= phase 1: projections =================
            with (
                tc.tile_pool(name="p1_in", bufs=1) as p1_in,
                tc.tile_pool(name="p1_w", bufs=1) as p1_w,
                tc.tile_pool(name="p1_kv", bufs=1) as p1_kv,
                tc.tile_pool(name="p1_sq", bufs=2) as p1_sq,
                tc.tile_pool(name="p1_ps", bufs=2, space="PSUM") as p1_ps,
                tc.tile_pool(name="p1_ss", bufs=1, space="PSUM") as p1_ss,
                tc.tile_pool(name="p1_f", bufs=2, space="PSUM") as p1_f,
            ):
                xT = [p1_in.tile([128, TN], F32R, tag=f"xT{k}", name=f"xTs{k}") for k in range(KT)]
                eT = [p1_in.tile([128, TN], F32R, tag=f"eT{k}", name=f"eTs{k}") for k in range(KT)]
                for k in range(KT):
                    nc.sync.dma_start(xT[k][:], xT_d.ap()[k * 128 : (k + 1) * 128, :])
                    nc.sync.dma_start(eT[k][:], encT_d.ap()[k * 128 : (k + 1) * 128, :])

                def qk_projection(w_d, in_tiles, bias_sb, scale_sb, out_tiles):
                    w_sb = p1_w.tile([128, KT, C], F32R, tag="W")
                    for k in range(KT):
                        nc.sync.dma_start(
                            w_sb[:, k, :], w_d.ap()[k * 128 : (k + 1) * 128, :]
                        )
                    ssum = p1_ss.tile([1, TN], F32, tag="ssum")
                    for m in range(MT):
                        ps = p1_ps.tile([128, TN], F32, tag="proj")
                        for k in range(KT):
                            nc.tensor.matmul(
                                ps[:],
                                w_sb[:, k, m * 128 : (m + 1) * 128],
                                in_tiles[k][:],
                                start=(k == 0),
                                stop=(k == KT - 1),
                            )
                        # bias add (psum f32 -> sbuf f32r)
                        nc.vector.tensor_scalar_add(
                            out_tiles[m][:], ps[:], bias_sb[:, m : m + 1]
                        )
                        sq = p1_sq.tile([128, TN], F32R, tag="sq")
                        nc.scalar.activation(sq[:], out_tiles[m][:], AF.Square)
                        nc.tensor.matmul(
                            ssum[:],
                            ones_col[:],
                            sq[:],
                            start=(m == 0),
                            stop=(m == MT - 1),
                        )
                    # rr = 1/sqrt(ssum/C + eps)
                    rms = p1_sq.tile([1, TN], F32, tag="rms")
                    nc.scalar.activation(
                        rms[:], ssum[:], AF.Sqrt, scale=1.0 / C, bias=eps_t[:]
                    )
                    rr = p1_sq.tile([1, TN], F32R, tag="rr")
                    with nc.allow_low_precision(reason="rms rsqrt broadcast"):
                        nc.vector.reciprocal(rr[:], rms[:])
                    for m in range(MT):
                        fps = p1_f.tile([128, TN], F32, tag="fps")
                        nc.tensor.matmul(
                            fps[:],
                            scale_sb[:, m * 128 : (m + 1) * 128],
                            rr[:],
                            start=True,
                            stop=True,
                        )
                        f1 = p1_sq.tile([128, TN], F32R, tag="f1")
                        nc.scalar.activation(f1[:], fps[:], AF.Copy, bias=1.0)
                        nc.vector.tensor_mul(out_tiles[m][:], out_tiles[m][:], f1[:])

                # q projection
                qk_projection(wq_d, xT, bq_sb, qs_sb, qT)

                # k projection -> kT tiles then bounce out
                kT = [p1_kv.tile([128, TN], F32R, tag=f"kT{m}", name=f"kTs{m}") for m in range(MT)]
                qk_projection(wk_d, eT, bk_sb, ks_sb, kT)
                for m in range(MT):
                    nc.sync.dma_start(
                        k_bounce.ap()[m * 128 : (m + 1) * 128, :], kT[m][:]
                    )

                # v projection: natural [s, c] layout, per-head padded + ones col
                v_loc = [
                    p1_kv.tile([128, H, DA], F32R, tag=f"vl{st}", name=f"vl{st}") for st in range(4)
                ]
                wv_sb = p1_w.tile([128, KT, C], F32R, tag="W")
                for k in range(KT):
                    nc.sync.dma_start(
                        wv_sb[:, k, :], wv_d.ap()[k * 128 : (k + 1) * 128, :]
                    )
                for st in range(4):
                    for jh in range(2):
                        ps = p1_ps.tile([128, TN], F32, tag="proj")
                        for k in range(KT):
                            nc.tensor.matmul(
                                ps[:],
                                eT[k][:, st * 128 : (st + 1) * 128],
                                wv_sb[:, k, jh * 512 : (jh + 1) * 512],
                                start=(k == 0),
                                stop=(k == KT - 1),
                            )
                        nc.vector.tensor_add(
                            v_loc[st][:, jh * 8 : (jh + 1) * 8, 0:D],
                            ps[:].rearrange("p (h d) -> p h d", h=8),
                            bv_bc[:, jh * 512 : (jh + 1) * 512].rearrange(
                                "p (h d) -> p h d", h=8
                            ),
                        )
                    nc.vector.tensor_copy(v_loc[st][:, :, D], ones_hf[:])
                    nc.sync.dma_start(
                        v_bounce.ap()[st * 128 : (st + 1) * 128, :],
                        v_loc[st][:].rearrange("p h d -> p (h d)"),
                    )

            # ================= phase 2: pairwise allgather =================
            nc.gpsimd.collective_compute(
                "AllGather",
                ALU.bypass,
                replica_groups=GROUPS,
                ins=[k_bounce.ap()],
                outs=[kg.ap()],
            )
            nc.gpsimd.collective_compute(
                "AllGather",
                ALU.bypass,
                replica_groups=GROUPS,
                ins=[v_bounce.ap()],
                outs=[vg.ap()],
            )

            # ================= phase 3: attention =================
            with (
                tc.tile_pool(name="p3_k", bufs=1) as p3_k,
                tc.tile_pool(name="p3_v", bufs=1) as p3_v,
                tc.tile_pool(name="p3_e", bufs=2) as p3_e,
                tc.tile_pool(name="p3_rb", bufs=2) as p3_rb,
                tc.tile_pool(name="p3_sc", bufs=3, space="PSUM") as p3_sc,
                tc.tile_pool(name="p3_y", bufs=2, space="PSUM") as p3_y,
                tc.tile_pool(name="p3_rp", bufs=2, space="PSUM") as p3_rp,
            ):
                kTf = [p3_k.tile([128, 2, TN], F32R, tag=f"kTf{m}", name=f"kTf{m}") for m in range(MT)]
                for m in range(MT):
                    nc.sync.dma_start(
                        kTf[m][:],
                        kg.ap()[:, m * 128 : (m + 1) * 128, :].transpose([1, 0, 2]),
                    )
                vf = [p3_v.tile([128, H, DA], F32R, tag=f"vf{j}", name=f"vf{j}") for j in range(ST)]
                for j in range(ST):
                    nc.sync.dma_start(
                        vf[j][:],
                        vg.ap()[j // 4, (j % 4) * 128 : (j % 4 + 1) * 128, :].rearrange(
                            "p (h d) -> p h d", h=H
                        ),
                    )

                for h in range(H):
                    mt, base = h // 2, (h % 2) * 64
                    qh = qT[mt][base : base + 64, :]
                    exps = []
                    for j in range(ST):
                        sc = p3_sc.tile([128, TN], F32, tag="sc")
                        nc.tensor.matmul(
                            sc[:],
                            kTf[mt][base : base + 64, j // 4,
                                    (j % 4) * 128 : (j % 4 + 1) * 128],
                            qh,
                            start=True,
                            stop=True,
                        )
                        ex = p3_e.tile([128, TN], F32R, tag=f"exp{j}")
                        nc.scalar.activation(ex[:], sc[:], AF.Exp, scale=float(D) ** -0.5)
                        exps.append(ex)
                    y_ps = p3_y.tile([DA, TN], F32, tag="y")
                    for j in range(ST):
                        nc.tensor.matmul(
                            y_ps[:],
                            vf[j][:, h, :],
                            exps[j][:],
                            start=(j == 0),
                            stop=(j == ST - 1),
                        )
                    recip = p3_rb.tile([1, TN], F32R, tag="recip")
                    with nc.allow_low_precision(reason="softmax 1/Z broadcast"):
                        nc.vector.reciprocal(recip[:], y_ps[64:65, :])
                    rb_ps = p3_rp.tile([128, TN], F32, tag="rb")
                    nc.tensor.matmul(
                        rb_ps[:], inv16_row[:], recip[:], start=True, stop=True
                    )
                    # yT[c-tile mt, rows base:base+64] = 16 * y_aug * rb
                    rb_sb = p3_rb.tile([128, TN], F32, tag="rbsb")
                    nc.scalar.activation(rb_sb[:], rb_ps[:], AF.Copy)
                    nc.vector.scalar_tensor_tensor(
                        yT[mt][base : base + 64, :],
                        y_ps[0:64, :],
                        16.0,
                        rb_sb[0:64, :],
                        ALU.mult,
                        ALU.mult,
                    )
                    # attn_mean accumulation: acc[j] += exp[j] * rb  (rb = 1/(16 Z))
                    for j in range(ST):
                        if h == 0:
                            nc.vector.tensor_mul(acc[j][:], exps[j][:], rb_ps[:])
                        else:
                            nc.vector.tensor_mul(exps[j][:], exps[j][:], rb_ps[:])
                            nc.vector.tensor_add(
                                acc[j][:], acc[j][:], exps[j][:].bitcast(F32)
                            )

            for j in range(ST):
                nc.sync.dma_start(am_o.ap()[j * 128 : (j + 1) * 128, :], acc[j][:])

            # ================= phase 4: output projection =================
            with (
                tc.tile_pool(name="p4_w", bufs=1) as p4_w,
                tc.tile_pool(name="p4_o", bufs=2) as p4_o,
                tc.tile_pool(name="p4_ps", bufs=2, space="PSUM") as p4_ps,
            ):
                wp_sb = p4_w.tile([128, KT, C], F32R, tag="Wp")
                for k in range(KT):
                    nc.sync.dma_start(
                        wp_sb[:, k, :], wp_d.ap()[k * 128 : (k + 1) * 128, :]
                    )
                for m in range(MT):
                    ps = p4_ps.tile([128, TN], F32, tag="yo")
                    for k in range(KT):
                        nc.tensor.matmul(
                            ps[:],
                            wp_sb[:, k, m * 128 : (m + 1) * 128],
                            yT[k][:],
                            start=(k == 0),
                            stop=(k == KT - 1),
                        )
                    yo = p4_o.tile([128, TN], F32, tag="yo_sb")
                    nc.vector.tensor_scalar_add(yo[:], ps[:], bp_sb[:, m : m + 1])
                    nc.sync.dma_start(yT_o.ap()[m * 128 : (m + 1) * 128, :], yo[:])

    nc.compile()
    return nc


_NC_CACHE = None


def _get_nc():
    global _NC_CACHE
    if _NC_CACHE is None:
        _NC_CACHE = build()
    return _NC_CACHE


def make_in_maps(x, encoder_output, Wq, bq, Wk, bk, Wv, bv, q_scale, k_scale,
                 Wp, bp):
    x = np.asarray(x, np.float32)
    enc = np.asarray(encoder_output, np.float32)
    Wq = np.ascontiguousarray(np.asarray(Wq, np.float32))
    Wk = np.ascontiguousarray(np.asarray(Wk, np.float32))
    Wv = np.ascontiguousarray(np.asarray(Wv, np.float32))
    Wp = np.ascontiguousarray(np.asarray(Wp, np.float32))
    bq_t = np.ascontiguousarray(np.asarray(bq, np.float32).reshape(MT, 128).T)
    bk_t = np.ascontiguousarray(np.asarray(bk, np.float32).reshape(MT, 128).T)
    bp_t = np.ascontiguousarray(np.asarray(bp, np.float32).reshape(MT, 128).T)
    bv = np.ascontiguousarray(np.asarray(bv, np.float32))
    qs = np.ascontiguousarray(np.asarray(q_scale, np.float32))
    ks = np.ascontiguousarray(np.asarray(k_scale, np.float32))

    in_maps = []
    for c in range(8):
        b, hf = c // 2, c % 2
        xT = np.ascontiguousarray(x[b, hf * TN : (hf + 1) * TN, :].T)
        encT = np.ascontiguousarray(enc[b, hf * TN : (hf + 1) * TN, :].T)
        in_maps.append(
            dict(xT=xT, encT=encT, wq=Wq, wk=Wk, wv=Wv, wp=Wp,
                 bq=bq_t, bk=bk_t, bp=bp_t, bv=bv, qs=qs, ks=ks)
        )

    return in_maps


def kernel(x, encoder_output, Wq, bq, Wk, bk, Wv, bv, q_scale, k_scale, Wp, bp,
           _trace=False):
    in_maps = make_in_maps(x, encoder_output, Wq, bq, Wk, bk, Wv, bv, q_scale,
                           k_scale, Wp, bp)
    nc = _get_nc()
    res = run_bass_kernel_spmd(nc, in_maps, core_ids=list(range(8)), trace=_trace)

    y = np.empty((B, T, C), np.float32)
    amean = np.empty((B, T, S), np.float32)
    for c in range(8):
        b, hf = c // 2, c % 2
        r = res.results[c]
        y[b, hf * TN : (hf + 1) * TN, :] = r["youtT"].T
        amean[b, hf * TN : (hf + 1) * TN, :] = r["ameanT"].T
    if _trace:
        kernel.last_exec_time_ns = res.exec_time_ns
        kernel.last_results = res
    return y, amean
